# revision 18
# baseline (speedup 1.0000x reference)
"""Trainium2 Bass kernel for nn_MultiHeadAttention (B=2, L=2048, D=1024, H=16).

Sharding: 8 cores = 2 batches (data parallel) x 4 head-groups (tensor
parallel, 4 heads / 256 features per core).  Each core computes its partial
o-proj output; the host sums the 4 partials per batch and adds the output
bias (the "all-reduce" of the unshard step).

Device-side layout is feature-major end to end so no on-device transposes
are needed: the host ships x already transposed to [d, t] and the weights
pre-sliced/transposed.  Matmul inputs are bitcast to float32r (full PE rate
for N>=256; plain fp32 runs at 1/4 rate).  Softmax denominators come for
free from an extra ones-column appended to V (row 64 of each AV psum
accumulator), so each PSUM bank hosts exactly one accumulation group.
"""

import os
import sys

import numpy as np

for _p in ("/opt/trn_rl_repo", "/root/.axon_site/_ro/trn_rl_repo"):
    if os.path.isdir(_p) and _p not in sys.path:
        sys.path.append(_p)

import concourse.bass as bass
import concourse.mybir as mybir
import concourse.tile as tile
from concourse import bacc
from concourse.bass_utils import run_bass_kernel_spmd

# Problem shape (hardcoded per contract)
B, L, D = 2, 2048, 1024
H, DH = 16, 64
N_CORES = 8
GROUPS = 4            # cores per batch (head-parallel)
HL = H // GROUPS      # 4 local heads per core
F = HL * DH           # 256 local features per core
KC = 128              # attention contraction chunk (k tokens)
NKC = L // KC         # 16 chunks
SPAN = 512            # matmul free-dim span
NSPAN = L // SPAN     # 4 spans
DC = 128              # projection contraction chunk
NDC = D // DC         # 8 chunks

F32 = mybir.dt.float32
F32R = mybir.dt.float32r
EXP = mybir.ActivationFunctionType.Exp


def build_bass(fix_waits=True):
    nc = bacc.Bacc("TRN2", target_bir_lowering=False)

    xq = nc.dram_tensor("xq", [128, NDC, L], F32R, kind="ExternalInput")
    xk = nc.dram_tensor("xk", [128, NDC, L], F32R, kind="ExternalInput")
    xv = nc.dram_tensor("xv", [128, NDC, L], F32R, kind="ExternalInput")
    wq = nc.dram_tensor("wq", [128, NDC, F], F32R, kind="ExternalInput")
    wk = nc.dram_tensor("wk", [128, NDC, F], F32R, kind="ExternalInput")
    wv = nc.dram_tensor("wv", [128, NDC, F], F32R, kind="ExternalInput")
    wo = nc.dram_tensor("wo", [64, 4, D], F32R, kind="ExternalInput")
    bqt = nc.dram_tensor("bqt", [128, 2], F32, kind="ExternalInput")
    bkt = nc.dram_tensor("bkt", [128, 2], F32, kind="ExternalInput")
    bvt = nc.dram_tensor("bvt", [1, F], F32, kind="ExternalInput")
    out = nc.dram_tensor("out", [L, D], F32, kind="ExternalOutput")
    scratch = nc.dram_tensor("scratch_recip", [2, 2, L], F32R)

    with tile.TileContext(nc) as tc:
        _emit(nc, tc, xq, xk, xv, wq, wk, wv, wo, bqt, bkt, bvt, out, scratch)
    # Bacc lowering: splits multi-wait sync_infos into EventSemaphores (the
    # walrus ISA structs have a single sync-wait slot), inserts gpsimd
    # library loads and ACT table loads.
    nc.compile()
    return nc


def _emit(nc, tc, xq, xk, xv, wq, wk, wv, wo, bqt, bkt, bvt, out, scratch):
    with (
        tc.tile_pool(name="consts", bufs=1) as consts,
        tc.tile_pool(name="weights", bufs=1) as weights,
        tc.tile_pool(name="persist", bufs=1) as persist,
        tc.tile_pool(name="xin", bufs=6) as xin,
        tc.tile_pool(name="ptp", bufs=3) as ptp,
        tc.tile_pool(name="rbp", bufs=2) as rbp,
        tc.tile_pool(name="outp", bufs=3) as outp,
        tc.tile_pool(name="mm", bufs=2, space="PSUM") as mmp,
        tc.tile_pool(name="acc", bufs=1, space="PSUM") as accp,
    ):
        # ---- constants ----
        ones_row = consts.tile([1, 128], F32, tag="ones_row", name="ones_row")
        nc.vector.memset(ones_row[:], 1.0)

        # ---- weights / biases to SBUF ----
        wq_sb = weights.tile([128, NDC, F], F32R, tag="wq", name="wq_sb")
        wk_sb = weights.tile([128, NDC, F], F32R, tag="wk", name="wk_sb")
        wv_sb = weights.tile([128, NDC, F], F32R, tag="wv", name="wv_sb")
        wo_sb = weights.tile([64, 4, D], F32R, tag="wo", name="wo_sb")
        nc.sync.dma_start(out=wq_sb[:], in_=wq[:])
        nc.sync.dma_start(out=wk_sb[:], in_=wk[:])
        nc.sync.dma_start(out=wv_sb[:], in_=wv[:])
        nc.sync.dma_start(out=wo_sb[:], in_=wo[:])
        bq_sb = consts.tile([128, 2], F32, tag="bq", name="bq_sb")
        bk_sb = consts.tile([128, 2], F32, tag="bk", name="bk_sb")
        bv_sb = consts.tile([1, F], F32, tag="bv", name="bv_sb")
        nc.sync.dma_start(out=bq_sb[:], in_=bqt[:])
        nc.sync.dma_start(out=bk_sb[:], in_=bkt[:])
        nc.sync.dma_start(out=bv_sb[:], in_=bvt[:])

        # ---- persistent activation tiles (per span, for fine-grained deps) ----
        # qhT/khT: [f(128 = 2 heads of pair), t] feature-major
        # vh: [t, 65] token-major per (pair, head, span); col 64 = ones (sums)
        qhT = {(p, s): persist.tile([128, SPAN], F32R, tag=f"qhT{p}{s}", name=f"qhT{p}{s}")
               for p in range(2) for s in range(NSPAN)}
        khT = {(p, s): persist.tile([128, SPAN], F32R, tag=f"khT{p}{s}", name=f"khT{p}{s}")
               for p in range(2) for s in range(NSPAN)}
        vh = {(p, h, s): persist.tile([128, 4, 65], F32R, tag=f"vh{p}{h}{s}", name=f"vh{p}{h}{s}")
              for p in range(2) for h in range(2) for s in range(NSPAN)}
        # avs: rows 0-63 = P @ [V|1] values, row 64 = softmax denominators
        avs = {(p, h): persist.tile([65, L], F32R, tag=f"avs{p}{h}", name=f"avs{p}{h}")
               for p in range(2) for h in range(2)}

        for key in vh:
            nc.vector.memset(vh[key][:, :, 64:65].bitcast(F32), 1.0)

        # ---- phase 1: projections, span by span ----
        for s in range(NSPAN):
            t0 = s * SPAN
            # DMA order matches consumption order (k-proj, q-proj, v-proj) so
            # FIFO DMA queues can never deadlock against xin slot recycling.
            xq_t, xk_t, xv_t = {}, {}, {}
            for nm, dram, dct in (("xk", xk, xk_t), ("xq", xq, xq_t),
                                  ("xv", xv, xv_t)):
                for o in range(NDC):
                    # v-proj re-reads every chunk for each of 4 psum tiles, so
                    # xv needs all 8 chunks resident to avoid slot deadlock
                    t = xin.tile([128, SPAN], F32R, tag=nm, name=nm,
                                 bufs=(8 if nm == "xv" else 6))
                    nc.sync.dma_start(out=t[:], in_=dram[:, o, t0:t0 + SPAN])
                    dct[o] = t

            # k then q proj: out [f 128, t 512] per pair
            for w_sb, x_t, b_sb, dst in (
                (wk_sb, xk_t, bk_sb, khT),
                (wq_sb, xq_t, bq_sb, qhT),
            ):
                for p in range(2):
                    ps = mmp.tile([128, 2 * SPAN], F32, tag="mm", name="mm")
                    for o in range(NDC):
                        nc.tensor.matmul(
                            ps[:, 0:SPAN],
                            (w_sb[:, o, p * 128:(p + 1) * 128]),
                            (x_t[o][:]),
                            start=(o == 0), stop=(o == NDC - 1),
                        )
                    # psum->sbuf copy with per-partition bias add
                    nc.vector.tensor_scalar_add(
                        dst[(p, s)][:], ps[:, 0:SPAN], b_sb[:, p:p + 1]
                    )

            # v proj: out [t 128, f 256] (both pairs at once), bias via K=1 matmul
            for tt in range(SPAN // 128):
                ps = mmp.tile([128, 2 * SPAN], F32, tag="mm", name="mm")
                for o in range(NDC):
                    nc.tensor.matmul(
                        ps[:, 0:F],
                        (xv_t[o][:, tt * 128:(tt + 1) * 128]),
                        (wv_sb[:, o, :]),
                        start=(o == 0), stop=False,
                    )
                nc.tensor.matmul(
                    ps[:, 0:F], ones_row[:], bv_sb[:],
                    start=False, stop=True,
                )
                for p in range(2):
                    for h in range(2):
                        nc.vector.tensor_copy(
                            vh[(p, h, s)][:, tt, 0:64],
                            ps[:, p * 128 + h * 64:p * 128 + (h + 1) * 64],
                        )

        # ---- phase 2: attention (flash-style over k chunks) ----
        for sp in range(2):            # q span-pairs of 1024
            q0 = sp * 2 * SPAN
            for p in range(2):         # head pairs AB / CD
                av = {h: accp.tile([128, 2 * SPAN], F32, tag=f"av{h}", name=f"av{h}")
                      for h in range(2)}
                for c in range(NKC):   # k chunks of 128
                    ks, cc = c // 4, c % 4
                    ko = cc * 128
                    st = {}
                    for h in range(2):  # heads in pair, row-packed QK^T
                        r0 = h * 64
                        stt = mmp.tile([128, 2 * SPAN], F32, tag="mm", name="mm")
                        for j in range(2):
                            nc.tensor.matmul(
                                stt[:, j * SPAN:(j + 1) * SPAN],
                                (khT[(p, ks)][r0:r0 + 64, ko:ko + 128]),
                                (qhT[(p, sp * 2 + j)][r0:r0 + 64, :]),
                                start=True, stop=True,
                                tile_position=(r0, 0),
                            )
                        st[h] = stt
                    pt = {}
                    for h in range(2):
                        ptt = ptp.tile([128, 2 * SPAN], F32R, tag="pt", name="pt")
                        nc.scalar.activation(ptt[:], st[h][:], EXP, scale=0.125)
                        pt[h] = ptt
                    for h in range(2):  # AV + fused denominators (row 64)
                        for j in range(2):
                            nc.tensor.matmul(
                                av[h][0:65, j * SPAN:(j + 1) * SPAN],
                                (vh[(p, h, ks)][:, cc, :]),
                                (pt[h][:, j * SPAN:(j + 1) * SPAN]),
                                start=(c == 0), stop=(c == NKC - 1),
                            )
                # drain accumulators (values + denominators) to SBUF
                for h in range(2):
                    nc.vector.tensor_copy(
                        avs[(p, h)][:, q0:q0 + 2 * SPAN], av[h][0:65, :]
                    )

        # ---- phase 3: normalize + o-proj ----
        for p in range(2):
            for h in range(2):
                a = avs[(p, h)]
                with nc.allow_low_precision(reason="softmax denominators are O(2048); fp32r rounding is ~1e-3 relative"):
                    nc.vector.reciprocal(a[64:65, :], a[64:65, :])
                # partition-broadcast the denominator row via a DRAM bounce
                # (SBUF sources cannot have 0-step partition dims)
                nc.sync.dma_start(out=scratch[p, h, :], in_=a[64:65, :])
                rb = rbp.tile([64, L], F32R, tag="rb", name="rb")
                nc.sync.dma_start(
                    out=rb[:], in_=scratch[p, h, :].partition_broadcast(64)
                )
                nc.vector.tensor_mul(a[0:64, :], a[0:64, :], rb[:])
        for tt in range(L // 128):
            o_sb = outp.tile([128, D], F32, tag="osb", name="osb")
            for m in range(2):
                o_ps = mmp.tile([128, 2 * SPAN], F32, tag="mm", name="mm")
                for i in range(4):
                    p, h = i // 2, i % 2
                    nc.tensor.matmul(
                        o_ps[:, 0:SPAN],
                        (avs[(p, h)][0:64, tt * 128:(tt + 1) * 128]),
                        (wo_sb[0:64, i, m * SPAN:(m + 1) * SPAN]),
                        start=(i == 0), stop=(i == 3),
                    )
                nc.vector.tensor_copy(o_sb[:, m * SPAN:(m + 1) * SPAN], o_ps[:, 0:SPAN])
            nc.sync.dma_start(out=out[tt * 128:(tt + 1) * 128, :], in_=o_sb[:])


def shard_inputs(q, k, v, Wq, bq, Wk, bk, Wv, bv, Wo, bo):
    """Host-side shard + layout prep. Returns list of 8 per-core input dicts."""
    def chunk_pf(a2d, pdim):
        # (n*pdim, f) -> (pdim, n, f) with row r = o*pdim + p
        n, f = a2d.shape
        return np.ascontiguousarray(
            a2d.reshape(n // pdim, pdim, f).transpose(1, 0, 2)
        ).astype(np.float32)

    in_maps = []
    for core in range(N_CORES):
        b = core // GROUPS
        g = core % GROUPS
        fs = slice(g * F, (g + 1) * F)
        m = {
            "xq": chunk_pf(np.ascontiguousarray(q[b].T), 128),
            "xk": chunk_pf(np.ascontiguousarray(k[b].T), 128),
            "xv": chunk_pf(np.ascontiguousarray(v[b].T), 128),
            "wq": chunk_pf(np.ascontiguousarray(Wq[fs, :].T), 128),
            "wk": chunk_pf(np.ascontiguousarray(Wk[fs, :].T), 128),
            "wv": chunk_pf(np.ascontiguousarray(Wv[fs, :].T), 128),
            "wo": chunk_pf(np.ascontiguousarray(Wo[:, fs].T), 64),
            "bqt": np.ascontiguousarray(bq[fs].reshape(2, 128).T).astype(np.float32),
            "bkt": np.ascontiguousarray(bk[fs].reshape(2, 128).T).astype(np.float32),
            "bvt": np.ascontiguousarray(bv[fs].reshape(1, F)).astype(np.float32),
        }
        in_maps.append(m)
    return in_maps


_NC_CACHE = None


def _get_nc():
    global _NC_CACHE
    if _NC_CACHE is None:
        _NC_CACHE = build_bass()
    return _NC_CACHE


def run_spmd(inputs, trace=False, **kw):
    """Run the 8-core kernel; returns (full_output, BassKernelResults)."""
    q = np.asarray(inputs["q"], np.float32)
    k = np.asarray(inputs["k"], np.float32)
    v = np.asarray(inputs["v"], np.float32)
    in_maps = shard_inputs(
        q, k, v,
        np.asarray(inputs["Wq"], np.float32), np.asarray(inputs["bq"], np.float32),
        np.asarray(inputs["Wk"], np.float32), np.asarray(inputs["bk"], np.float32),
        np.asarray(inputs["Wv"], np.float32), np.asarray(inputs["bv"], np.float32),
        np.asarray(inputs["Wo"], np.float32), np.asarray(inputs["bo"], np.float32),
    )
    nc = _get_nc()
    res = run_bass_kernel_spmd(nc, in_maps, core_ids=list(range(N_CORES)),
                               trace=trace, **kw)
    bo = np.asarray(inputs["bo"], np.float32)
    full = np.empty((B, L, D), np.float32)
    for b in range(B):
        acc = res.results[b * GROUPS]["out"].astype(np.float32)
        for g in range(1, GROUPS):
            acc = acc + res.results[b * GROUPS + g]["out"]
        full[b] = acc + bo[None, :]
    return full, res


def kernel(**inputs):
    kpm = np.asarray(inputs["key_padding_mask"])
    if not bool(kpm.all()):
        return _numpy_fallback(**inputs)
    out, _ = run_spmd(inputs)
    return out


def _numpy_fallback(q, k, v, key_padding_mask, Wq, bq, Wk, bk, Wv, bv, Wo, bo):
    q = np.asarray(q, np.float32)
    k = np.asarray(k, np.float32)
    v = np.asarray(v, np.float32)
    B_, Lq, D_ = q.shape
    qh = (q @ np.asarray(Wq).T + bq).reshape(B_, Lq, H, DH).transpose(0, 2, 1, 3)
    kh = (k @ np.asarray(Wk).T + bk).reshape(B_, -1, H, DH).transpose(0, 2, 1, 3)
    vh = (v @ np.asarray(Wv).T + bv).reshape(B_, -1, H, DH).transpose(0, 2, 1, 3)
    s = np.einsum("bhqd,bhkd->bhqk", qh, kh) / np.sqrt(np.float32(DH))
    km = np.asarray(key_padding_mask)[:, None, None, :]
    s = np.where(km, s, -np.inf)
    s = s - s.max(-1, keepdims=True)
    p = np.exp(s)
    p = p / p.sum(-1, keepdims=True)
    o = np.einsum("bhqk,bhkd->bhqd", p, vh)
    o = o.transpose(0, 2, 1, 3).reshape(B_, Lq, D_)
    return (o @ np.asarray(Wo).T + bo).astype(np.float32)
